# revision 4
# baseline (speedup 1.0000x reference)
"""Trainium2 Bass kernel for nn_AttentionSheafLearner.

Computation:  maps = x[row] @ W[:, :C].T + x[col] @ W[:, C:].T    [E, 25]
              out  = eye(5) - softmax(maps.reshape(E, 5, 5), axis=-1)

Strategy (8 NeuronCores, SPMD):
  - Precompute z[n] = [x[n] @ Wr.T | x[n] @ Wc.T | pad]  (128 bf16 = 256B rows)
    on device with bf16 PE matmuls; store per node-half tables in DRAM.
  - Edges are sharded by VALUE class: nodes split in two halves (A = <25088),
    edge class = (row_half, col_half); each of the 4 classes is handled by 2
    cores. This keeps per-core gather indices < 25088 so they fit int16, the
    index dtype of the SWDGE dma_gather instruction.
  - Per 7296-edge chunk: dma_gather 256B z rows for row and col endpoints,
    DVE add -> maps, ScalarE exp, DVE strided reduce / reciprocal /
    broadcast-mul -> sm (bf16), store sm.
  - Host computes out = eye - sm and re-permutes rows to original edge order.
"""

import os

import numpy as np

# problem sizes (hardcoded per contract)
N = 50000
C = 128
D = 5
DD = D * D          # 25
E = 1_600_000
NCORES = 8
P = 128

HALF = 25088        # nodes per half (padded; 2*HALF >= N)
ZW = 128            # z row width in bf16 (256B, dma_gather elem size)
NCH_H = HALF // P   # 196 node chunks per half

T = 1596            # edge cols per partition per core => capacity 204288/core
EPC = P * T         # 204288
CHKC = 114          # edge cols per partition per chunk
CHK = P * CHKC      # 7296 edges per chunk
NCHUNK = T // CHKC  # 28
IDXW = CHK // 16    # 456 idx cols per chunk
IDX_COLS = NCHUNK * IDXW  # 12768

_XBLK = 28          # node chunks per xT DMA block
_ZGRP = 14          # node chunks per z store group


def _build_nc():
    from contextlib import ExitStack

    import concourse.bacc as bacc
    import concourse.mybir as mybir
    import concourse.tile as tile

    f32 = mybir.dt.float32
    bf16 = mybir.dt.bfloat16
    i16 = mybir.dt.int16

    nc = bacc.Bacc(
        "TRN2",
        target_bir_lowering=False,
        debug=False,
        enable_asserts=False,
        num_devices=NCORES,
        num_swdge_queues=4,
        dynamic_dma_scratch_size=16384,
    )

    xt_r_d = nc.dram_tensor("xt_r", [P, HALF], bf16, kind="ExternalInput")
    xt_c_d = nc.dram_tensor("xt_c", [P, HALF], bf16, kind="ExternalInput")
    w_d = nc.dram_tensor("w", [P, 2 * DD], bf16, kind="ExternalInput")
    ridx_d = nc.dram_tensor("ridx", [P, IDX_COLS], i16, kind="ExternalInput")
    cidx_d = nc.dram_tensor("cidx", [P, IDX_COLS], i16, kind="ExternalInput")
    z_r_d = nc.dram_tensor("z_r", [HALF, ZW], bf16)
    z_c_d = nc.dram_tensor("z_c", [HALF, ZW], bf16)
    out_d = nc.dram_tensor("out", [EPC, DD], bf16, kind="ExternalOutput")

    oview = out_d.ap().rearrange("(p t) d -> p (t d)", p=P)  # [128, T*25]

    with tile.TileContext(nc) as tc, ExitStack() as ctx:
        const_pool = ctx.enter_context(tc.tile_pool(name="const", bufs=1))
        w_tile = const_pool.tile([P, 2 * DD], bf16)
        nc.sync.dma_start(w_tile[:], w_d.ap())

        # ---- stage A: z tables ----
        with ExitStack() as actx:
            xt_pool = actx.enter_context(tc.tile_pool(name="xt", bufs=2))
            z_pool = actx.enter_context(tc.tile_pool(name="zsb", bufs=3))
            ps_pool = actx.enter_context(
                tc.tile_pool(name="ps", bufs=4, space="PSUM")
            )
            for xt_d, z_d in ((xt_r_d, z_r_d), (xt_c_d, z_c_d)):
                zview = z_d.ap().rearrange("(i p) d -> i p d", p=P)
                for blk in range(NCH_H // _XBLK):  # 7
                    xt_tile = xt_pool.tile([P, _XBLK * P], bf16)
                    nc.sync.dma_start(
                        xt_tile[:],
                        xt_d.ap()[:, blk * _XBLK * P:(blk + 1) * _XBLK * P],
                    )
                    for grp in range(_XBLK // _ZGRP):  # 2
                        z_sb = z_pool.tile([P, _ZGRP * ZW], bf16)
                        for j in range(_ZGRP):
                            jj = grp * _ZGRP + j
                            ps = ps_pool.tile([P, 2 * DD], f32, space="PSUM")
                            nc.tensor.matmul(
                                ps[:],
                                xt_tile[:, jj * P:(jj + 1) * P],
                                w_tile[:],
                                start=True,
                                stop=True,
                            )
                            # pad cols [2*DD:ZW] are never read downstream
                            nc.scalar.copy(
                                z_sb[:, j * ZW:j * ZW + 2 * DD], ps[:]
                            )
                        i0 = blk * _XBLK + grp * _ZGRP
                        nc.sync.dma_start(
                            zview[i0:i0 + _ZGRP, :, :].rearrange("i p d -> p i d"),
                            z_sb[:].rearrange("p (i d) -> p i d", i=_ZGRP),
                        )

        # ---- stage B: gather + softmax ----
        g_pool = ctx.enter_context(tc.tile_pool(name="g", bufs=2))
        i_pool = ctx.enter_context(tc.tile_pool(name="ix", bufs=2))
        m_pool = ctx.enter_context(tc.tile_pool(name="m", bufs=2))
        e_pool = ctx.enter_context(tc.tile_pool(name="e", bufs=2))
        s_pool = ctx.enter_context(tc.tile_pool(name="s", bufs=2))
        o_pool = ctx.enter_context(tc.tile_pool(name="o", bufs=2))
        for ch in range(NCHUNK):
            ri = i_pool.tile([P, IDXW], i16, tag="ri")
            nc.sync.dma_start(ri[:], ridx_d.ap()[:, ch * IDXW:(ch + 1) * IDXW])
            ci = i_pool.tile([P, IDXW], i16, tag="ci")
            nc.sync.dma_start(ci[:], cidx_d.ap()[:, ch * IDXW:(ch + 1) * IDXW])
            g_r = g_pool.tile([P, CHKC * ZW], bf16, tag="gr")
            nc.gpsimd.dma_gather(
                out_ap=g_r[:].rearrange("p (u d) -> p u d", d=ZW),
                in_ap=z_r_d.ap(),
                idxs_ap=ri[:],
                num_idxs=CHK,
                num_idxs_reg=CHK,
                elem_size=ZW,
                single_packet=False,
                queue_num=(2 * ch) % 4,
            )
            g_c = g_pool.tile([P, CHKC * ZW], bf16, tag="gc")
            nc.gpsimd.dma_gather(
                out_ap=g_c[:].rearrange("p (u d) -> p u d", d=ZW),
                in_ap=z_c_d.ap(),
                idxs_ap=ci[:],
                num_idxs=CHK,
                num_idxs_reg=CHK,
                elem_size=ZW,
                single_packet=False,
                queue_num=(2 * ch + 1) % 4,
            )
            m = m_pool.tile([P, CHKC * DD], f32)
            nc.vector.tensor_tensor(
                out=m[:].rearrange("p (u d) -> p u d", d=DD),
                in0=g_r[:].rearrange("p (u d) -> p u d", d=ZW)[:, :, 0:DD],
                in1=g_c[:].rearrange("p (u d) -> p u d", d=ZW)[:, :, DD:2 * DD],
                op=mybir.AluOpType.add,
            )
            et = e_pool.tile([P, CHKC * DD], f32)
            nc.scalar.activation(et[:], m[:], mybir.ActivationFunctionType.Exp)
            e3 = et[:].rearrange("p (t d) -> p t d", d=D)  # [128, CHKC*5, 5]
            s = s_pool.tile([P, CHKC * D], f32, tag="s")
            nc.vector.reduce_sum(s[:], e3, axis=mybir.AxisListType.X)
            r = s_pool.tile([P, CHKC * D], f32, tag="r")
            nc.vector.reciprocal(r[:], s[:])
            o = o_pool.tile([P, CHKC * DD], bf16)
            nc.vector.tensor_tensor(
                out=o[:].rearrange("p (t d) -> p t d", d=D),
                in0=e3,
                in1=r[:].unsqueeze(2).to_broadcast([P, CHKC * D, D]),
                op=mybir.AluOpType.mult,
            )
            nc.sync.dma_start(oview[:, ch * CHKC * DD:(ch + 1) * CHKC * DD], o[:])

    nc.compile()
    return nc


def _host_prep(x, W, edge_index):
    """Shard edges by (row_half, col_half) class across cores; build per-core
    inputs. Returns (in_maps, slot_maps, spill) where slot_maps[c] holds the
    original edge id for each real (non-pad) list position, and spill is a
    list of original edge ids handled on host."""
    x = np.asarray(x, dtype=np.float32)
    W = np.asarray(W, dtype=np.float32)
    ei = np.asarray(edge_index)
    row = ei[0].astype(np.int64)
    col = ei[1].astype(np.int64)

    xt = np.zeros((P, 2 * HALF), dtype=np.float32)
    xt[:, :N] = x.T
    xtb = xt.astype(np.bfloat16) if hasattr(np, "bfloat16") else None
    if xtb is None:
        import ml_dtypes

        xtb = xt.astype(ml_dtypes.bfloat16)
    xt_half = [
        np.ascontiguousarray(xtb[:, :HALF]),
        np.ascontiguousarray(xtb[:, HALF:]),
    ]

    w = np.zeros((P, 2 * DD), dtype=np.float32)
    w[:, :DD] = W[:, :C].T
    w[:, DD:2 * DD] = W[:, C:].T
    w = w.astype(xtb.dtype)

    cls = (row >= HALF).astype(np.int64) * 2 + (col >= HALF)
    order = np.argsort(cls, kind="stable")
    counts = np.bincount(cls, minlength=4)
    starts = np.concatenate([[0], np.cumsum(counts)])

    in_maps = []
    slot_maps = []
    spill = []
    for core in range(NCORES):
        k = core // 2
        half_r, half_c = k >> 1, k & 1
        cls_edges = order[starts[k]:starts[k + 1]]
        sub = cls_edges[core % 2::2]          # interleave class across 2 cores
        if len(sub) > EPC:
            spill.extend(sub[EPC:].tolist())
            sub = sub[:EPC]
        m = len(sub)
        lr = np.zeros(EPC, dtype=np.int16)
        lc = np.zeros(EPC, dtype=np.int16)
        lr[:m] = (row[sub] - half_r * HALF).astype(np.int16)
        lc[:m] = (col[sub] - half_c * HALF).astype(np.int16)

        def wrap(loc):
            # list position l = ch*CHK + p*CHKC + u  ->  gather pos i = u*128+p
            a = loc.reshape(NCHUNK, P, CHKC).transpose(0, 2, 1)  # [ch, u, p]
            a = a.reshape(NCHUNK, CHK)                           # gather order
            a = a.reshape(NCHUNK, IDXW, 16)
            a = a.transpose(2, 0, 1).reshape(16, IDX_COLS)       # [16, cols]
            return np.ascontiguousarray(np.tile(a, (8, 1)))

        in_maps.append(
            {
                "xt_r": xt_half[half_r],
                "xt_c": xt_half[half_c],
                "w": w,
                "ridx": wrap(lr),
                "cidx": wrap(lc),
            }
        )
        slot_maps.append(sub)
    return in_maps, slot_maps, spill


def _host_spill_compute(x, W, edge_index, ids):
    row = np.asarray(edge_index[0])[ids].astype(np.int64)
    col = np.asarray(edge_index[1])[ids].astype(np.int64)
    x = np.asarray(x, dtype=np.float32)
    W = np.asarray(W, dtype=np.float32)
    maps = (x[row] @ W[:, :C].T + x[col] @ W[:, C:].T).reshape(-1, D, D)
    em = np.exp(maps - maps.max(-1, keepdims=True))
    sm = em / em.sum(-1, keepdims=True)
    return np.eye(D, dtype=np.float32)[None] - sm


LAST_EXEC_NS = None


def kernel(x, W, edge_index):
    global LAST_EXEC_NS
    from concourse.bass_utils import run_bass_kernel_spmd

    nc = _build_nc()
    in_maps, slot_maps, spill = _host_prep(x, W, edge_index)
    trace = os.environ.get("KERNEL_TRACE", "0") == "1"
    br = run_bass_kernel_spmd(
        nc,
        in_maps,
        core_ids=list(range(NCORES)),
        trace=trace,
    )
    LAST_EXEC_NS = br.exec_time_ns

    eye_flat = np.eye(D, dtype=np.float32).reshape(1, DD)
    out = np.empty((E, DD), dtype=np.float32)
    for core in range(NCORES):
        res = np.asarray(br.results[core]["out"], dtype=np.float32)  # [EPC,25]
        ordered = (
            res.reshape(P, NCHUNK, CHKC, DD)
            .transpose(1, 0, 2, 3)
            .reshape(EPC, DD)
        )                                              # list-position order
        ids = slot_maps[core]
        out[ids] = eye_flat - ordered[: len(ids)]
    if spill:
        out[np.asarray(spill)] = _host_spill_compute(
            x, W, edge_index, np.asarray(spill)
        ).reshape(-1, DD)
    return out.reshape(E, D, D).astype(np.float32)


# revision 5
# speedup vs baseline: 1.0732x; 1.0732x over previous
"""Trainium2 Bass kernel for nn_AttentionSheafLearner.

Computation:  maps = x[row] @ W[:, :C].T + x[col] @ W[:, C:].T    [E, 25]
              out  = eye(5) - softmax(maps.reshape(E, 5, 5), axis=-1)

Strategy (8 NeuronCores, SPMD):
  - Precompute z[n] = [x[n] @ Wr.T | x[n] @ Wc.T | pad]  (128 bf16 = 256B rows)
    on device with bf16 PE matmuls; store per node-half tables in DRAM.
  - Edges are sharded by VALUE class: nodes split in two halves (A = <25088),
    edge class = (row_half, col_half); each of the 4 classes is handled by 2
    cores (keeps gather indices < 25088 so they fit int16).
  - Same-row edges are grouped (K in {8,4,2,1} members per group) so ONE
    row-side gather descriptor serves K edges: group g -> partition g%128,
    super-chunk s=g//128.  Col side gathers one 256B row per edge.  This cuts
    SWDGE descriptors per edge from 2.0 to ~1.26 (the drain of 256B gather
    packets at ~17ns/packet/engine is the kernel's bottleneck).
  - maps = broadcast-add (DVE, stride-0 over K), ScalarE exp, DVE reduce /
    reciprocal / broadcast-mul -> sm (bf16), store sm.
  - Host computes out = eye - sm and scatters rows to original edge order.
"""

import math
import os

import numpy as np

# problem sizes (hardcoded per contract)
N = 50000
C = 128
D = 5
DD = D * D          # 25
E = 1_600_000
NCORES = 8
P = 128

HALF = 25088        # nodes per half (padded; 2*HALF >= N)
ZW = 128            # z row width in bf16 (256B, dma_gather elem size)
NCH_H = HALF // P   # 196 node chunks per half

_XBLK = 28          # node chunks per xT DMA block
_ZGRP = 14          # node chunks per z store group

KS = (8, 4, 2, 1)
# slot-cols per compute tile, per K region (tile = S_t super-chunks, K*S_t cols)
SLOTC = {8: 64, 4: 64, 2: 64, 1: 32}


def _build_nc(scs):
    """scs: {K: super-chunk count} region capacities (128 groups per SC)."""
    from contextlib import ExitStack

    import concourse.bacc as bacc
    import concourse.mybir as mybir
    import concourse.tile as tile

    f32 = mybir.dt.float32
    bf16 = mybir.dt.bfloat16
    i16 = mybir.dt.int16

    tcol = sum(K * scs[K] for K in KS)            # out slot-cols per partition
    rpos = sum(scs[K] * P for K in KS)            # row idx positions
    cpos = tcol * P                               # col idx positions

    nc = bacc.Bacc(
        "TRN2",
        target_bir_lowering=False,
        debug=False,
        enable_asserts=False,
        num_devices=NCORES,
        num_swdge_queues=4,
    )

    xt_r_d = nc.dram_tensor("xt_r", [P, HALF], bf16, kind="ExternalInput")
    xt_c_d = nc.dram_tensor("xt_c", [P, HALF], bf16, kind="ExternalInput")
    w_d = nc.dram_tensor("w", [P, 2 * DD], bf16, kind="ExternalInput")
    ridx_d = nc.dram_tensor("ridx", [P, rpos // 16], i16, kind="ExternalInput")
    cidx_d = nc.dram_tensor("cidx", [P, cpos // 16], i16, kind="ExternalInput")
    z_r_d = nc.dram_tensor("z_r", [HALF, ZW], bf16)
    z_c_d = nc.dram_tensor("z_c", [HALF, ZW], bf16)
    out_d = nc.dram_tensor("out", [P, tcol * DD], bf16, kind="ExternalOutput")

    with tile.TileContext(nc) as tc, ExitStack() as ctx:
        const_pool = ctx.enter_context(tc.tile_pool(name="const", bufs=1))
        w_tile = const_pool.tile([P, 2 * DD], bf16)
        nc.sync.dma_start(w_tile[:], w_d.ap())

        # ---- stage A: z tables ----
        with ExitStack() as actx:
            xt_pool = actx.enter_context(tc.tile_pool(name="xt", bufs=2))
            z_pool = actx.enter_context(tc.tile_pool(name="zsb", bufs=3))
            ps_pool = actx.enter_context(
                tc.tile_pool(name="ps", bufs=4, space="PSUM")
            )
            for xt_d, z_d in ((xt_r_d, z_r_d), (xt_c_d, z_c_d)):
                zview = z_d.ap().rearrange("(i p) d -> i p d", p=P)
                for blk in range(NCH_H // _XBLK):  # 7
                    xt_tile = xt_pool.tile([P, _XBLK * P], bf16)
                    nc.sync.dma_start(
                        xt_tile[:],
                        xt_d.ap()[:, blk * _XBLK * P:(blk + 1) * _XBLK * P],
                    )
                    for grp in range(_XBLK // _ZGRP):  # 2
                        z_sb = z_pool.tile([P, _ZGRP * ZW], bf16)
                        for j in range(_ZGRP):
                            jj = grp * _ZGRP + j
                            ps = ps_pool.tile([P, 2 * DD], f32, space="PSUM")
                            nc.tensor.matmul(
                                ps[:],
                                xt_tile[:, jj * P:(jj + 1) * P],
                                w_tile[:],
                                start=True,
                                stop=True,
                            )
                            # pad cols [2*DD:ZW] are never read downstream
                            nc.scalar.copy(
                                z_sb[:, j * ZW:j * ZW + 2 * DD], ps[:]
                            )
                        i0 = blk * _XBLK + grp * _ZGRP
                        nc.sync.dma_start(
                            zview[i0:i0 + _ZGRP, :, :].rearrange("i p d -> p i d"),
                            z_sb[:].rearrange("p (i d) -> p i d", i=_ZGRP),
                        )

        # ---- stage B: grouped gathers + softmax ----
        gc_pool = ctx.enter_context(tc.tile_pool(name="gc", bufs=4))
        gr_pool = ctx.enter_context(tc.tile_pool(name="gr", bufs=4))
        i_pool = ctx.enter_context(tc.tile_pool(name="ix", bufs=4))
        m_pool = ctx.enter_context(tc.tile_pool(name="m", bufs=3))
        e_pool = ctx.enter_context(tc.tile_pool(name="e", bufs=3))
        s_pool = ctx.enter_context(tc.tile_pool(name="s", bufs=2))
        o_pool = ctx.enter_context(tc.tile_pool(name="o", bufs=3))

        rbase = 0   # row idx position base
        cbase = 0   # col idx / slot position base (slot-col = cbase//P)
        qi = 0
        for K in KS:
            SC = scs[K]
            S_t = SLOTC[K] // K
            for t in range(math.ceil(SC / S_t)):
                sct = min(S_t, SC - t * S_t)      # super-chunks this tile
                slotc = sct * K                   # slot-cols this tile
                nrow = sct * P
                ncol = slotc * P
                ri = i_pool.tile([P, nrow // 16], i16, tag="ri")
                nc.sync.dma_start(
                    ri[:], ridx_d.ap()[:, rbase // 16:(rbase + nrow) // 16]
                )
                ci = i_pool.tile([P, ncol // 16], i16, tag="ci")
                nc.sync.dma_start(
                    ci[:], cidx_d.ap()[:, cbase // 16:(cbase + ncol) // 16]
                )
                g_r = gr_pool.tile([P, sct * ZW], bf16, tag="gr")
                nc.gpsimd.dma_gather(
                    out_ap=g_r[:].rearrange("p (s d) -> p s d", d=ZW),
                    in_ap=z_r_d.ap(),
                    idxs_ap=ri[:],
                    num_idxs=nrow,
                    num_idxs_reg=nrow,
                    elem_size=ZW,
                    single_packet=False,
                    queue_num=qi % 4,
                )
                qi += 1
                g_c = gc_pool.tile([P, slotc * ZW], bf16, tag="gc")
                nc.gpsimd.dma_gather(
                    out_ap=g_c[:].rearrange("p (u d) -> p u d", d=ZW),
                    in_ap=z_c_d.ap(),
                    idxs_ap=ci[:],
                    num_idxs=ncol,
                    num_idxs_reg=ncol,
                    elem_size=ZW,
                    single_packet=False,
                    queue_num=qi % 4,
                )
                qi += 1
                m = m_pool.tile([P, slotc * DD], f32)
                nc.vector.tensor_tensor(
                    out=m[:].rearrange("p (s k d) -> p s k d", k=K, d=DD),
                    in0=g_r[:].rearrange("p (s d) -> p s d", d=ZW)[:, :, 0:DD]
                    .unsqueeze(2)
                    .to_broadcast([P, sct, K, DD]),
                    in1=g_c[:].rearrange("p (s k d) -> p s k d", k=K, d=ZW)[
                        :, :, :, DD:2 * DD
                    ],
                    op=mybir.AluOpType.add,
                )
                et = e_pool.tile([P, slotc * DD], f32)
                nc.scalar.activation(
                    et[:], m[:], mybir.ActivationFunctionType.Exp
                )
                e3 = et[:].rearrange("p (t d) -> p t d", d=D)
                s = s_pool.tile([P, slotc * D], f32, tag="s")
                nc.vector.reduce_sum(s[:], e3, axis=mybir.AxisListType.X)
                r = s_pool.tile([P, slotc * D], f32, tag="r")
                nc.vector.reciprocal(r[:], s[:])
                o = o_pool.tile([P, slotc * DD], bf16)
                nc.vector.tensor_tensor(
                    out=o[:].rearrange("p (t d) -> p t d", d=D),
                    in0=e3,
                    in1=r[:].unsqueeze(2).to_broadcast([P, slotc * D, D]),
                    op=mybir.AluOpType.mult,
                )
                coff = cbase // P
                nc.sync.dma_start(
                    out_d.ap()[:, coff * DD:(coff + slotc) * DD], o[:]
                )
                rbase += nrow
                cbase += ncol

    nc.compile()
    return nc, tcol


def _wrap16(stream):
    """Gather idx layout: position i -> [i%16, i//16], replicated to 128."""
    a = stream.reshape(-1, 16).T                       # [16, L/16]
    return np.ascontiguousarray(np.tile(a, (8, 1)))    # [128, L/16]


def _pack_core(lr, lc, eids):
    """Group same-row edges into K in {8,4,2,1} sized groups (all full)."""
    ordr = np.argsort(lr, kind="stable")
    lr_s = lr[ordr]
    deg = np.bincount(lr_s, minlength=HALF)
    node_start = np.concatenate([[0], np.cumsum(deg)])
    n8 = deg // 8
    r = deg % 8
    has4 = (r >= 4).astype(np.int64)
    has2 = ((r % 4) >= 2).astype(np.int64)
    has1 = r % 2
    packs = {}
    for K, base_off in (
        (8, None),
        (4, 8 * n8),
        (2, 8 * n8 + 4 * has4),
        (1, 8 * n8 + 4 * has4 + 2 * has2),
    ):
        if K == 8:
            nodes = np.repeat(np.arange(HALF), n8)
            j = np.arange(len(nodes)) - np.repeat(
                np.concatenate([[0], np.cumsum(n8)])[:-1], n8
            )
            offs = node_start[nodes] + 8 * j
        else:
            cnt = {4: has4, 2: has2, 1: has1}[K]
            nodes = np.nonzero(cnt)[0]
            offs = node_start[nodes] + base_off[nodes]
        mem = offs[:, None] + np.arange(K)[None, :]
        eidx = ordr[mem]                                # [G, K] edge positions
        packs[K] = (
            nodes.astype(np.int16),
            lc[eidx].astype(np.int16),
            eids[eidx],
        )
    return packs


def _host_prep(x, W, edge_index):
    x = np.asarray(x, dtype=np.float32)
    W = np.asarray(W, dtype=np.float32)
    ei = np.asarray(edge_index)
    row = ei[0].astype(np.int64)
    col = ei[1].astype(np.int64)

    try:
        bf = np.dtype("bfloat16")
    except TypeError:
        import ml_dtypes

        bf = np.dtype(ml_dtypes.bfloat16)
    xt = np.zeros((P, 2 * HALF), dtype=np.float32)
    xt[:, :N] = x.T
    xtb = xt.astype(bf)
    xt_half = [
        np.ascontiguousarray(xtb[:, :HALF]),
        np.ascontiguousarray(xtb[:, HALF:]),
    ]
    w = np.zeros((P, 2 * DD), dtype=np.float32)
    w[:, :DD] = W[:, :C].T
    w[:, DD:2 * DD] = W[:, C:].T
    w = w.astype(bf)

    cls = (row >= HALF).astype(np.int64) * 2 + (col >= HALF)
    order = np.argsort(cls, kind="stable")
    counts = np.bincount(cls, minlength=4)
    starts = np.concatenate([[0], np.cumsum(counts)])

    core_packs = []
    for core in range(NCORES):
        k = core // 2
        half_r, half_c = k >> 1, k & 1
        cls_edges = order[starts[k]:starts[k + 1]]
        sub = cls_edges[core % 2::2]
        lr = (row[sub] - half_r * HALF).astype(np.int32)
        lc = (col[sub] - half_c * HALF).astype(np.int32)
        core_packs.append(_pack_core(lr, lc, sub))

    # region capacities: cross-core max groups, rounded to full super-chunks
    scs = {
        K: (max(len(p[K][0]) for p in core_packs) + P - 1) // P for K in KS
    }
    tcol = sum(K * scs[K] for K in KS)

    in_maps = []
    slot_maps = []
    for core in range(NCORES):
        packs = core_packs[core]
        k = core // 2
        half_r, half_c = k >> 1, k & 1
        rstreams, cstreams = [], []
        slot_eid = np.full((P, tcol), -1, dtype=np.int64)
        coff = 0
        for K in KS:
            G_cap = scs[K] * P
            nodes, cols_, eids = packs[K]
            G = len(nodes)
            npad = np.zeros(G_cap, dtype=np.int16)
            npad[:G] = nodes
            rstreams.append(npad)
            cpad = np.zeros((G_cap, K), dtype=np.int16)
            cpad[:G] = cols_
            # position i = (s*K + k)*128 + p for group g=(s,p): [SC,128,K]->[SC,K,128]
            cstreams.append(
                np.ascontiguousarray(
                    cpad.reshape(scs[K], P, K).transpose(0, 2, 1)
                ).reshape(-1)
            )
            epad = np.full((G_cap, K), -1, dtype=np.int64)
            epad[:G] = eids
            slot_eid[:, coff:coff + scs[K] * K] = (
                epad.reshape(scs[K], P, K).transpose(1, 0, 2).reshape(P, -1)
            )
            coff += scs[K] * K
        in_maps.append(
            {
                "xt_r": xt_half[half_r],
                "xt_c": xt_half[half_c],
                "w": w,
                "ridx": _wrap16(np.concatenate(rstreams)),
                "cidx": _wrap16(np.concatenate(cstreams)),
            }
        )
        slot_maps.append(slot_eid)
    return in_maps, slot_maps, scs, tcol


LAST_EXEC_NS = None


def kernel(x, W, edge_index):
    global LAST_EXEC_NS
    from concourse.bass_utils import run_bass_kernel_spmd

    in_maps, slot_maps, scs, tcol = _host_prep(x, W, edge_index)
    nc, tcol_b = _build_nc(scs)
    assert tcol_b == tcol
    trace = os.environ.get("KERNEL_TRACE", "0") == "1"
    br = run_bass_kernel_spmd(
        nc,
        in_maps,
        core_ids=list(range(NCORES)),
        trace=trace,
    )
    LAST_EXEC_NS = br.exec_time_ns

    eye_flat = np.eye(D, dtype=np.float32).reshape(1, DD)
    out = np.empty((E, DD), dtype=np.float32)
    for core in range(NCORES):
        res = np.asarray(br.results[core]["out"], dtype=np.float32)
        res = res.reshape(P, tcol, DD)
        ids = slot_maps[core]                     # [P, tcol]
        valid = ids >= 0
        out[ids[valid]] = eye_flat - res[valid]
    return out.reshape(E, D, D).astype(np.float32)


# revision 9
# speedup vs baseline: 1.4546x; 1.3554x over previous
"""Trainium2 Bass kernel for nn_AttentionSheafLearner.

Computation:  maps = x[row] @ W[:, :C].T + x[col] @ W[:, C:].T    [E, 25]
              out  = eye(5) - softmax(maps.reshape(E, 5, 5), axis=-1)

Strategy (8 NeuronCores, SPMD):
  - Precompute z[n] = [x[n] @ Wr.T | x[n] @ Wc.T | pad]  (128 bf16 = 256B rows)
    on device with bf16 PE matmuls; store per node-half tables in DRAM.
  - Edges are sharded by VALUE class: nodes split in two halves (A = <25088),
    edge class = (row_half, col_half); each of the 4 classes is handled by 2
    cores (keeps gather indices < 25088 so they fit int16).
  - Same-row edges are grouped (K in {8,4,2,1} members per group) so ONE
    row-side gather descriptor serves K edges: group g -> partition g%128,
    super-chunk s=g//128.  Col side gathers one 256B row per edge.  This cuts
    SWDGE descriptors per edge from 2.0 to ~1.26 (the drain of 256B gather
    packets at ~17ns/packet/engine is the kernel's bottleneck).
  - maps = broadcast-add (DVE, stride-0 over K), ScalarE exp, DVE reduce /
    reciprocal / broadcast-mul -> sm (bf16), store sm.
  - Host computes out = eye - sm and scatters rows to original edge order.
"""

import math
import os

import numpy as np

# problem sizes (hardcoded per contract)
N = 50000
C = 128
D = 5
DD = D * D          # 25
E = 1_600_000
NCORES = 8
P = 128

HALF = 25088        # nodes per half (padded; 2*HALF >= N)
ZW = 128            # z row width in bf16 (256B, dma_gather elem size)
NCH_H = HALF // P   # 196 node chunks per half

_XBLK = 28          # node chunks per xT DMA block
_ZGRP = 14          # node chunks per z store group

KS = (8, 4, 2, 1)
# slot-cols per compute tile, per K region (tile = S_t super-chunks, K*S_t cols)
SLOTC = {8: 64, 4: 64, 2: 64, 1: 32}


def _build_nc(scs):
    """scs: {K: super-chunk count} region capacities (128 groups per SC)."""
    from contextlib import ExitStack

    import concourse.bacc as bacc
    import concourse.mybir as mybir
    import concourse.tile as tile

    f32 = mybir.dt.float32
    bf16 = mybir.dt.bfloat16
    i16 = mybir.dt.int16

    tcol = sum(K * scs[K] for K in KS)            # out slot-cols per partition
    rpos = sum(scs[K] * P for K in KS)            # row idx positions
    cpos = tcol * P                               # col idx positions

    nc = bacc.Bacc(
        "TRN2",
        target_bir_lowering=False,
        debug=False,
        enable_asserts=False,
        num_devices=NCORES,
        num_swdge_queues=4,
    )

    xt_r_d = nc.dram_tensor("xt_r", [P, HALF], bf16, kind="ExternalInput")
    xt_c_d = nc.dram_tensor("xt_c", [P, HALF], bf16, kind="ExternalInput")
    w_d = nc.dram_tensor("w", [P, 2 * DD], bf16, kind="ExternalInput")
    ridx_d = nc.dram_tensor("ridx", [P, rpos // 16], i16, kind="ExternalInput")
    cidx_d = nc.dram_tensor("cidx", [P, cpos // 16], i16, kind="ExternalInput")
    z_r_d = nc.dram_tensor("z_r", [HALF, ZW], bf16)
    z_c_d = nc.dram_tensor("z_c", [HALF, ZW], bf16)
    out_d = nc.dram_tensor("out", [P, tcol * DD], bf16, kind="ExternalOutput")

    with tile.TileContext(nc) as tc, ExitStack() as ctx:
        const_pool = ctx.enter_context(tc.tile_pool(name="const", bufs=1))
        w_tile = const_pool.tile([P, 2 * DD], bf16)
        nc.sync.dma_start(w_tile[:], w_d.ap())

        # ---- stage A: z tables ----
        with ExitStack() as actx:
            xt_pool = actx.enter_context(tc.tile_pool(name="xt", bufs=2))
            z_pool = actx.enter_context(tc.tile_pool(name="zsb", bufs=3))
            ps_pool = actx.enter_context(
                tc.tile_pool(name="ps", bufs=4, space="PSUM")
            )
            for xt_d, z_d in ((xt_c_d, z_c_d), (xt_r_d, z_r_d)):
                zview = z_d.ap().rearrange("(i p) d -> i p d", p=P)
                for blk in range(NCH_H // _XBLK):  # 7
                    xt_tile = xt_pool.tile([P, _XBLK * P], bf16)
                    nc.sync.dma_start(
                        xt_tile[:],
                        xt_d.ap()[:, blk * _XBLK * P:(blk + 1) * _XBLK * P],
                    )
                    for grp in range(_XBLK // _ZGRP):  # 2
                        z_sb = z_pool.tile([P, _ZGRP * ZW], bf16)
                        for j in range(_ZGRP):
                            jj = grp * _ZGRP + j
                            ps = ps_pool.tile([P, 2 * DD], f32, space="PSUM")
                            nc.tensor.matmul(
                                ps[:],
                                xt_tile[:, jj * P:(jj + 1) * P],
                                w_tile[:],
                                start=True,
                                stop=True,
                            )
                            # pad cols [2*DD:ZW] are never read downstream
                            nc.scalar.copy(
                                z_sb[:, j * ZW:j * ZW + 2 * DD], ps[:]
                            )
                        i0 = blk * _XBLK + grp * _ZGRP
                        nc.sync.dma_start(
                            zview[i0:i0 + _ZGRP, :, :].rearrange("i p d -> p i d"),
                            z_sb[:].rearrange("p (i d) -> p i d", i=_ZGRP),
                        )

        # ---- stage B: grouped gathers + softmax ----
        # Row gathers: ONE instruction per K-region (the row side of region K
        # is only scs[K]*128 descriptors); its output tile stays resident
        # while the region's col tiles consume it.  Col gathers: one per
        # compute tile, 4-queue round-robin.
        gc_pool = ctx.enter_context(tc.tile_pool(name="gc", bufs=4))
        gr_pool = ctx.enter_context(tc.tile_pool(name="gr", bufs=2))
        i_pool = ctx.enter_context(tc.tile_pool(name="ix", bufs=4))
        ir_pool = ctx.enter_context(tc.tile_pool(name="ixr", bufs=2))
        m_pool = ctx.enter_context(tc.tile_pool(name="m", bufs=3))
        e_pool = ctx.enter_context(tc.tile_pool(name="e", bufs=3))
        s_pool = ctx.enter_context(tc.tile_pool(name="s", bufs=2))
        o_pool = ctx.enter_context(tc.tile_pool(name="o", bufs=3))

        rbase = 0   # row idx position base
        cbase = 0   # col idx / slot position base (slot-col = cbase//P)
        qi = 0
        grmax = max(scs.values())
        for K in KS:
            SC = scs[K]
            S_t = SLOTC[K] // K
            nrow = SC * P
            ri = ir_pool.tile([P, nrow // 16], i16, tag="ri")
            nc.sync.dma_start(
                ri[:], ridx_d.ap()[:, rbase // 16:(rbase + nrow) // 16]
            )
            g_r = gr_pool.tile([P, grmax * ZW], bf16, tag="gr")
            nc.gpsimd.dma_gather(
                out_ap=g_r[:, :SC * ZW].rearrange("p (s d) -> p s d", d=ZW),
                in_ap=z_r_d.ap(),
                idxs_ap=ri[:],
                num_idxs=nrow,
                num_idxs_reg=nrow,
                elem_size=ZW,
                single_packet=False,
                queue_num=qi % 4,
            )
            qi += 1
            for t in range(math.ceil(SC / S_t)):
                s0 = t * S_t
                sct = min(S_t, SC - s0)           # super-chunks this tile
                slotc = sct * K                   # slot-cols this tile
                ncol = slotc * P
                ci = i_pool.tile([P, ncol // 16], i16, tag="ci")
                nc.sync.dma_start(
                    ci[:], cidx_d.ap()[:, cbase // 16:(cbase + ncol) // 16]
                )
                g_c = gc_pool.tile([P, slotc * ZW], bf16, tag="gc")
                nc.gpsimd.dma_gather(
                    out_ap=g_c[:].rearrange("p (u d) -> p u d", d=ZW),
                    in_ap=z_c_d.ap(),
                    idxs_ap=ci[:],
                    num_idxs=ncol,
                    num_idxs_reg=ncol,
                    elem_size=ZW,
                    single_packet=False,
                    queue_num=qi % 4,
                )
                qi += 1
                m = m_pool.tile([P, slotc * DD], f32)
                nc.vector.tensor_tensor(
                    out=m[:].rearrange("p (s k d) -> p s k d", k=K, d=DD),
                    in0=g_r[:].rearrange("p (s d) -> p s d", d=ZW)[
                        :, s0:s0 + sct, 0:DD
                    ]
                    .unsqueeze(2)
                    .to_broadcast([P, sct, K, DD]),
                    in1=g_c[:].rearrange("p (s k d) -> p s k d", k=K, d=ZW)[
                        :, :, :, DD:2 * DD
                    ],
                    op=mybir.AluOpType.add,
                )
                et = e_pool.tile([P, slotc * DD], f32)
                nc.scalar.activation(
                    et[:], m[:], mybir.ActivationFunctionType.Exp
                )
                e3 = et[:].rearrange("p (t d) -> p t d", d=D)
                s = s_pool.tile([P, slotc * D], f32, tag="s")
                nc.vector.reduce_sum(s[:], e3, axis=mybir.AxisListType.X)
                r = s_pool.tile([P, slotc * D], f32, tag="r")
                nc.vector.reciprocal(r[:], s[:])
                o = o_pool.tile([P, slotc * DD], bf16)
                nc.vector.tensor_tensor(
                    out=o[:].rearrange("p (t d) -> p t d", d=D),
                    in0=e3,
                    in1=r[:].unsqueeze(2).to_broadcast([P, slotc * D, D]),
                    op=mybir.AluOpType.mult,
                )
                coff = cbase // P
                nc.sync.dma_start(
                    out_d.ap()[:, coff * DD:(coff + slotc) * DD], o[:]
                )
                cbase += ncol
            rbase += nrow

    nc.compile()
    return nc, tcol


def _wrap16(stream):
    """Gather idx layout: position i -> [i%16, i//16], replicated to 128."""
    a = stream.reshape(-1, 16).T                       # [16, L/16]
    return np.ascontiguousarray(np.tile(a, (8, 1)))    # [128, L/16]


def _pack_core(lr, lc, eids):
    """Group same-row edges into K in {8,4,2,1} sized groups (all full)."""
    ordr = np.argsort(lr, kind="stable")
    lr_s = lr[ordr]
    deg = np.bincount(lr_s, minlength=HALF)
    node_start = np.concatenate([[0], np.cumsum(deg)])
    n8 = deg // 8
    r = deg % 8
    has4 = (r >= 4).astype(np.int64)
    has2 = ((r % 4) >= 2).astype(np.int64)
    has1 = r % 2
    packs = {}
    for K, base_off in (
        (8, None),
        (4, 8 * n8),
        (2, 8 * n8 + 4 * has4),
        (1, 8 * n8 + 4 * has4 + 2 * has2),
    ):
        if K == 8:
            nodes = np.repeat(np.arange(HALF), n8)
            j = np.arange(len(nodes)) - np.repeat(
                np.concatenate([[0], np.cumsum(n8)])[:-1], n8
            )
            offs = node_start[nodes] + 8 * j
        else:
            cnt = {4: has4, 2: has2, 1: has1}[K]
            nodes = np.nonzero(cnt)[0]
            offs = node_start[nodes] + base_off[nodes]
        mem = offs[:, None] + np.arange(K)[None, :]
        eidx = ordr[mem]                                # [G, K] edge positions
        packs[K] = (
            nodes.astype(np.int16),
            lc[eidx].astype(np.int16),
            eids[eidx],
        )
    return packs


def _host_prep(x, W, edge_index):
    x = np.asarray(x, dtype=np.float32)
    W = np.asarray(W, dtype=np.float32)
    ei = np.asarray(edge_index)
    row = ei[0].astype(np.int64)
    col = ei[1].astype(np.int64)

    try:
        bf = np.dtype("bfloat16")
    except TypeError:
        import ml_dtypes

        bf = np.dtype(ml_dtypes.bfloat16)
    xt = np.zeros((P, 2 * HALF), dtype=np.float32)
    xt[:, :N] = x.T
    xtb = xt.astype(bf)
    xt_half = [
        np.ascontiguousarray(xtb[:, :HALF]),
        np.ascontiguousarray(xtb[:, HALF:]),
    ]
    w = np.zeros((P, 2 * DD), dtype=np.float32)
    w[:, :DD] = W[:, :C].T
    w[:, DD:2 * DD] = W[:, C:].T
    w = w.astype(bf)

    cls = (row >= HALF).astype(np.int64) * 2 + (col >= HALF)
    order = np.argsort(cls, kind="stable")
    counts = np.bincount(cls, minlength=4)
    starts = np.concatenate([[0], np.cumsum(counts)])

    core_packs = []
    for core in range(NCORES):
        k = core // 2
        half_r, half_c = k >> 1, k & 1
        cls_edges = order[starts[k]:starts[k + 1]]
        sub = cls_edges[core % 2::2]
        lr = (row[sub] - half_r * HALF).astype(np.int32)
        lc = (col[sub] - half_c * HALF).astype(np.int32)
        core_packs.append(_pack_core(lr, lc, sub))

    # region capacities: cross-core max groups, rounded to full super-chunks
    scs = {
        K: (max(len(p[K][0]) for p in core_packs) + P - 1) // P for K in KS
    }
    tcol = sum(K * scs[K] for K in KS)

    in_maps = []
    slot_maps = []
    for core in range(NCORES):
        packs = core_packs[core]
        k = core // 2
        half_r, half_c = k >> 1, k & 1
        rstreams, cstreams = [], []
        slot_eid = np.full((P, tcol), -1, dtype=np.int64)
        coff = 0
        for K in KS:
            G_cap = scs[K] * P
            nodes, cols_, eids = packs[K]
            G = len(nodes)
            npad = np.zeros(G_cap, dtype=np.int16)
            npad[:G] = nodes
            rstreams.append(npad)
            cpad = np.zeros((G_cap, K), dtype=np.int16)
            cpad[:G] = cols_
            # position i = (s*K + k)*128 + p for group g=(s,p): [SC,128,K]->[SC,K,128]
            cstreams.append(
                np.ascontiguousarray(
                    cpad.reshape(scs[K], P, K).transpose(0, 2, 1)
                ).reshape(-1)
            )
            epad = np.full((G_cap, K), -1, dtype=np.int64)
            epad[:G] = eids
            slot_eid[:, coff:coff + scs[K] * K] = (
                epad.reshape(scs[K], P, K).transpose(1, 0, 2).reshape(P, -1)
            )
            coff += scs[K] * K
        in_maps.append(
            {
                "xt_r": xt_half[half_r],
                "xt_c": xt_half[half_c],
                "w": w,
                "ridx": _wrap16(np.concatenate(rstreams)),
                "cidx": _wrap16(np.concatenate(cstreams)),
            }
        )
        slot_maps.append(slot_eid)
    return in_maps, slot_maps, scs, tcol


LAST_EXEC_NS = None


def kernel(x, W, edge_index):
    global LAST_EXEC_NS
    from concourse.bass_utils import run_bass_kernel_spmd

    in_maps, slot_maps, scs, tcol = _host_prep(x, W, edge_index)
    nc, tcol_b = _build_nc(scs)
    assert tcol_b == tcol
    trace = os.environ.get("KERNEL_TRACE", "0") == "1"
    br = run_bass_kernel_spmd(
        nc,
        in_maps,
        core_ids=list(range(NCORES)),
        trace=trace,
    )
    LAST_EXEC_NS = br.exec_time_ns

    eye_flat = np.eye(D, dtype=np.float32).reshape(1, DD)
    out = np.empty((E, DD), dtype=np.float32)
    for core in range(NCORES):
        res = np.asarray(br.results[core]["out"], dtype=np.float32)
        res = res.reshape(P, tcol, DD)
        ids = slot_maps[core]                     # [P, tcol]
        valid = ids >= 0
        out[ids[valid]] = eye_flat - res[valid]
    return out.reshape(E, D, D).astype(np.float32)


# revision 10
# speedup vs baseline: 1.4988x; 1.0304x over previous
"""Trainium2 Bass kernel for nn_AttentionSheafLearner.

Computation:  maps = x[row] @ W[:, :C].T + x[col] @ W[:, C:].T    [E, 25]
              out  = eye(5) - softmax(maps.reshape(E, 5, 5), axis=-1)

Strategy (8 NeuronCores, SPMD):
  - Precompute z[n] = [x[n] @ Wr.T | x[n] @ Wc.T | pad]  (128 bf16 = 256B rows)
    on device with bf16 PE matmuls; store per node-half tables in DRAM.
  - Edges are sharded by VALUE class: nodes split in two halves (A = <25088),
    edge class = (row_half, col_half); each of the 4 classes is handled by 2
    cores (keeps gather indices < 25088 so they fit int16).
  - Same-row edges are grouped (K in {8,4,2,1} members per group) so ONE
    row-side gather descriptor serves K edges: group g -> partition g%128,
    super-chunk s=g//128.  Col side gathers one 256B row per edge.  This cuts
    SWDGE descriptors per edge from 2.0 to ~1.26 (the drain of 256B gather
    packets at ~17ns/packet/engine is the kernel's bottleneck).
  - maps = broadcast-add (DVE, stride-0 over K), ScalarE exp, DVE reduce /
    reciprocal / broadcast-mul -> sm (bf16), store sm.
  - Host computes out = eye - sm and scatters rows to original edge order.
"""

import math
import os

import numpy as np

# problem sizes (hardcoded per contract)
N = 50000
C = 128
D = 5
DD = D * D          # 25
E = 1_600_000
NCORES = 8
P = 128

HALF = 25088        # nodes per half (padded; 2*HALF >= N)
ZW = 128            # z row width in bf16 (256B, dma_gather elem size)
NCH_H = HALF // P   # 196 node chunks per half

_XBLK = 28          # node chunks per xT DMA block
_ZGRP = 14          # node chunks per z store group

KS = (8, 4, 2, 1)
# slot-cols per compute tile, per K region (tile = S_t super-chunks, K*S_t cols)
SLOTC = {8: 64, 4: 64, 2: 64, 1: 32}


def _build_nc(scs):
    """scs: {K: super-chunk count} region capacities (128 groups per SC)."""
    from contextlib import ExitStack

    import concourse.bacc as bacc
    import concourse.mybir as mybir
    import concourse.tile as tile

    f32 = mybir.dt.float32
    bf16 = mybir.dt.bfloat16
    i16 = mybir.dt.int16

    tcol = sum(K * scs[K] for K in KS)            # out slot-cols per partition
    rpos = sum(scs[K] * P for K in KS)            # row idx positions
    cpos = tcol * P                               # col idx positions

    nc = bacc.Bacc(
        "TRN2",
        target_bir_lowering=False,
        debug=False,
        enable_asserts=False,
        num_devices=NCORES,
        num_swdge_queues=4,
    )

    xt_r_d = nc.dram_tensor("xt_r", [P, HALF], bf16, kind="ExternalInput")
    xt_c_d = nc.dram_tensor("xt_c", [P, HALF], bf16, kind="ExternalInput")
    w_d = nc.dram_tensor("w", [P, 2 * DD], bf16, kind="ExternalInput")
    ridx_d = nc.dram_tensor("ridx", [P, rpos // 16], i16, kind="ExternalInput")
    cidx_d = nc.dram_tensor("cidx", [P, cpos // 16], i16, kind="ExternalInput")
    z_r_d = nc.dram_tensor("z_r", [HALF, ZW], bf16)
    z_c_d = nc.dram_tensor("z_c", [HALF, ZW], bf16)
    out_d = nc.dram_tensor("out", [P, tcol * DD], bf16, kind="ExternalOutput")

    with tile.TileContext(nc) as tc, ExitStack() as ctx:
        const_pool = ctx.enter_context(tc.tile_pool(name="const", bufs=1))
        w_tile = const_pool.tile([P, 2 * DD], bf16)
        nc.sync.dma_start(w_tile[:], w_d.ap())

        # ---- stage A: z tables ----
        with ExitStack() as actx:
            xt_pool = actx.enter_context(tc.tile_pool(name="xt", bufs=2))
            z_pool = actx.enter_context(tc.tile_pool(name="zsb", bufs=3))
            ps_pool = actx.enter_context(
                tc.tile_pool(name="ps", bufs=4, space="PSUM")
            )
            for xt_d, z_d in ((xt_c_d, z_c_d), (xt_r_d, z_r_d)):
                zview = z_d.ap().rearrange("(i p) d -> i p d", p=P)
                for blk in range(NCH_H // _XBLK):  # 7
                    xt_tile = xt_pool.tile([P, _XBLK * P], bf16)
                    nc.sync.dma_start(
                        xt_tile[:],
                        xt_d.ap()[:, blk * _XBLK * P:(blk + 1) * _XBLK * P],
                    )
                    for grp in range(_XBLK // _ZGRP):  # 2
                        z_sb = z_pool.tile([P, _ZGRP * ZW], bf16)
                        for j in range(_ZGRP):
                            jj = grp * _ZGRP + j
                            ps = ps_pool.tile([P, 2 * DD], f32, space="PSUM")
                            nc.tensor.matmul(
                                ps[:],
                                xt_tile[:, jj * P:(jj + 1) * P],
                                w_tile[:],
                                start=True,
                                stop=True,
                            )
                            # pad cols [2*DD:ZW] are never read downstream
                            nc.scalar.copy(
                                z_sb[:, j * ZW:j * ZW + 2 * DD], ps[:]
                            )
                        i0 = blk * _XBLK + grp * _ZGRP
                        nc.sync.dma_start(
                            zview[i0:i0 + _ZGRP, :, :].rearrange("i p d -> p i d"),
                            z_sb[:].rearrange("p (i d) -> p i d", i=_ZGRP),
                        )

        # ---- stage B: grouped gathers + softmax ----
        # Row gathers: ONE instruction per K-region (the row side of region K
        # is only scs[K]*128 descriptors); its output tile stays resident
        # while the region's col tiles consume it.  Col gathers: one per
        # compute tile, 4-queue round-robin.
        gc_pool = ctx.enter_context(tc.tile_pool(name="gc", bufs=6))
        gr_pool = ctx.enter_context(tc.tile_pool(name="gr", bufs=2))
        i_pool = ctx.enter_context(tc.tile_pool(name="ix", bufs=4))
        ir_pool = ctx.enter_context(tc.tile_pool(name="ixr", bufs=2))
        m_pool = ctx.enter_context(tc.tile_pool(name="m", bufs=2))
        e_pool = ctx.enter_context(tc.tile_pool(name="e", bufs=2))
        s_pool = ctx.enter_context(tc.tile_pool(name="s", bufs=2))
        o_pool = ctx.enter_context(tc.tile_pool(name="o", bufs=2))

        rbase = 0   # row idx position base
        cbase = 0   # col idx / slot position base (slot-col = cbase//P)
        qi = 0
        grmax = max(scs.values())
        for K in KS:
            SC = scs[K]
            S_t = SLOTC[K] // K
            nrow = SC * P
            ri = ir_pool.tile([P, nrow // 16], i16, tag="ri")
            nc.sync.dma_start(
                ri[:], ridx_d.ap()[:, rbase // 16:(rbase + nrow) // 16]
            )
            g_r = gr_pool.tile([P, grmax * ZW], bf16, tag="gr")
            nc.gpsimd.dma_gather(
                out_ap=g_r[:, :SC * ZW].rearrange("p (s d) -> p s d", d=ZW),
                in_ap=z_r_d.ap(),
                idxs_ap=ri[:],
                num_idxs=nrow,
                num_idxs_reg=nrow,
                elem_size=ZW,
                single_packet=False,
                queue_num=qi % 4,
            )
            qi += 1
            for t in range(math.ceil(SC / S_t)):
                s0 = t * S_t
                sct = min(S_t, SC - s0)           # super-chunks this tile
                slotc = sct * K                   # slot-cols this tile
                ncol = slotc * P
                ci = i_pool.tile([P, ncol // 16], i16, tag="ci")
                nc.sync.dma_start(
                    ci[:], cidx_d.ap()[:, cbase // 16:(cbase + ncol) // 16]
                )
                g_c = gc_pool.tile([P, slotc * ZW], bf16, tag="gc")
                nc.gpsimd.dma_gather(
                    out_ap=g_c[:].rearrange("p (u d) -> p u d", d=ZW),
                    in_ap=z_c_d.ap(),
                    idxs_ap=ci[:],
                    num_idxs=ncol,
                    num_idxs_reg=ncol,
                    elem_size=ZW,
                    single_packet=False,
                    queue_num=qi % 4,
                )
                qi += 1
                m = m_pool.tile([P, slotc * DD], f32)
                nc.vector.tensor_tensor(
                    out=m[:].rearrange("p (s k d) -> p s k d", k=K, d=DD),
                    in0=g_r[:].rearrange("p (s d) -> p s d", d=ZW)[
                        :, s0:s0 + sct, 0:DD
                    ]
                    .unsqueeze(2)
                    .to_broadcast([P, sct, K, DD]),
                    in1=g_c[:].rearrange("p (s k d) -> p s k d", k=K, d=ZW)[
                        :, :, :, DD:2 * DD
                    ],
                    op=mybir.AluOpType.add,
                )
                et = e_pool.tile([P, slotc * DD], f32)
                nc.scalar.activation(
                    et[:], m[:], mybir.ActivationFunctionType.Exp
                )
                e3 = et[:].rearrange("p (t d) -> p t d", d=D)
                s = s_pool.tile([P, slotc * D], f32, tag="s")
                nc.vector.reduce_sum(s[:], e3, axis=mybir.AxisListType.X)
                r = s_pool.tile([P, slotc * D], f32, tag="r")
                nc.vector.reciprocal(r[:], s[:])
                o = o_pool.tile([P, slotc * DD], bf16)
                nc.vector.tensor_tensor(
                    out=o[:].rearrange("p (t d) -> p t d", d=D),
                    in0=e3,
                    in1=r[:].unsqueeze(2).to_broadcast([P, slotc * D, D]),
                    op=mybir.AluOpType.mult,
                )
                coff = cbase // P
                nc.sync.dma_start(
                    out_d.ap()[:, coff * DD:(coff + slotc) * DD], o[:]
                )
                cbase += ncol
            rbase += nrow

    nc.compile()
    return nc, tcol


def _wrap16(stream):
    """Gather idx layout: position i -> [i%16, i//16], replicated to 128."""
    a = stream.reshape(-1, 16).T                       # [16, L/16]
    return np.ascontiguousarray(np.tile(a, (8, 1)))    # [128, L/16]


def _pack_core(lr, lc, eids):
    """Group same-row edges into K in {8,4,2,1} sized groups (all full)."""
    ordr = np.argsort(lr, kind="stable")
    lr_s = lr[ordr]
    deg = np.bincount(lr_s, minlength=HALF)
    node_start = np.concatenate([[0], np.cumsum(deg)])
    n8 = deg // 8
    r = deg % 8
    has4 = (r >= 4).astype(np.int64)
    has2 = ((r % 4) >= 2).astype(np.int64)
    has1 = r % 2
    packs = {}
    for K, base_off in (
        (8, None),
        (4, 8 * n8),
        (2, 8 * n8 + 4 * has4),
        (1, 8 * n8 + 4 * has4 + 2 * has2),
    ):
        if K == 8:
            nodes = np.repeat(np.arange(HALF), n8)
            j = np.arange(len(nodes)) - np.repeat(
                np.concatenate([[0], np.cumsum(n8)])[:-1], n8
            )
            offs = node_start[nodes] + 8 * j
        else:
            cnt = {4: has4, 2: has2, 1: has1}[K]
            nodes = np.nonzero(cnt)[0]
            offs = node_start[nodes] + base_off[nodes]
        mem = offs[:, None] + np.arange(K)[None, :]
        eidx = ordr[mem]                                # [G, K] edge positions
        packs[K] = (
            nodes.astype(np.int16),
            lc[eidx].astype(np.int16),
            eids[eidx],
        )
    return packs


def _host_prep(x, W, edge_index):
    x = np.asarray(x, dtype=np.float32)
    W = np.asarray(W, dtype=np.float32)
    ei = np.asarray(edge_index)
    row = ei[0].astype(np.int64)
    col = ei[1].astype(np.int64)

    try:
        bf = np.dtype("bfloat16")
    except TypeError:
        import ml_dtypes

        bf = np.dtype(ml_dtypes.bfloat16)
    xt = np.zeros((P, 2 * HALF), dtype=np.float32)
    xt[:, :N] = x.T
    xtb = xt.astype(bf)
    xt_half = [
        np.ascontiguousarray(xtb[:, :HALF]),
        np.ascontiguousarray(xtb[:, HALF:]),
    ]
    w = np.zeros((P, 2 * DD), dtype=np.float32)
    w[:, :DD] = W[:, :C].T
    w[:, DD:2 * DD] = W[:, C:].T
    w = w.astype(bf)

    cls = (row >= HALF).astype(np.int64) * 2 + (col >= HALF)
    order = np.argsort(cls, kind="stable")
    counts = np.bincount(cls, minlength=4)
    starts = np.concatenate([[0], np.cumsum(counts)])

    core_packs = []
    for core in range(NCORES):
        k = core // 2
        half_r, half_c = k >> 1, k & 1
        cls_edges = order[starts[k]:starts[k + 1]]
        sub = cls_edges[core % 2::2]
        lr = (row[sub] - half_r * HALF).astype(np.int32)
        lc = (col[sub] - half_c * HALF).astype(np.int32)
        core_packs.append(_pack_core(lr, lc, sub))

    # region capacities: cross-core max groups, rounded to full super-chunks
    scs = {
        K: (max(len(p[K][0]) for p in core_packs) + P - 1) // P for K in KS
    }
    tcol = sum(K * scs[K] for K in KS)

    in_maps = []
    slot_maps = []
    for core in range(NCORES):
        packs = core_packs[core]
        k = core // 2
        half_r, half_c = k >> 1, k & 1
        rstreams, cstreams = [], []
        slot_eid = np.full((P, tcol), -1, dtype=np.int64)
        coff = 0
        for K in KS:
            G_cap = scs[K] * P
            nodes, cols_, eids = packs[K]
            G = len(nodes)
            npad = np.zeros(G_cap, dtype=np.int16)
            npad[:G] = nodes
            rstreams.append(npad)
            cpad = np.zeros((G_cap, K), dtype=np.int16)
            cpad[:G] = cols_
            # position i = (s*K + k)*128 + p for group g=(s,p): [SC,128,K]->[SC,K,128]
            cstreams.append(
                np.ascontiguousarray(
                    cpad.reshape(scs[K], P, K).transpose(0, 2, 1)
                ).reshape(-1)
            )
            epad = np.full((G_cap, K), -1, dtype=np.int64)
            epad[:G] = eids
            slot_eid[:, coff:coff + scs[K] * K] = (
                epad.reshape(scs[K], P, K).transpose(1, 0, 2).reshape(P, -1)
            )
            coff += scs[K] * K
        in_maps.append(
            {
                "xt_r": xt_half[half_r],
                "xt_c": xt_half[half_c],
                "w": w,
                "ridx": _wrap16(np.concatenate(rstreams)),
                "cidx": _wrap16(np.concatenate(cstreams)),
            }
        )
        slot_maps.append(slot_eid)
    return in_maps, slot_maps, scs, tcol


LAST_EXEC_NS = None


def kernel(x, W, edge_index):
    global LAST_EXEC_NS
    from concourse.bass_utils import run_bass_kernel_spmd

    in_maps, slot_maps, scs, tcol = _host_prep(x, W, edge_index)
    nc, tcol_b = _build_nc(scs)
    assert tcol_b == tcol
    trace = os.environ.get("KERNEL_TRACE", "0") == "1"
    br = run_bass_kernel_spmd(
        nc,
        in_maps,
        core_ids=list(range(NCORES)),
        trace=trace,
    )
    LAST_EXEC_NS = br.exec_time_ns

    eye_flat = np.eye(D, dtype=np.float32).reshape(1, DD)
    out = np.empty((E, DD), dtype=np.float32)
    for core in range(NCORES):
        res = np.asarray(br.results[core]["out"], dtype=np.float32)
        res = res.reshape(P, tcol, DD)
        ids = slot_maps[core]                     # [P, tcol]
        valid = ids >= 0
        out[ids[valid]] = eye_flat - res[valid]
    return out.reshape(E, D, D).astype(np.float32)
